# revision 15
# baseline (speedup 1.0000x reference)
"""Bayesian dense MoE (top-2 of 8 experts) on 8 Trainium2 NeuronCores.

Math (per reference):
    logits = x @ gk + gb                      [B, E]
    gw     = renorm-top2(softmax(logits))     [B, E]   (softmax denom cancels)
    se     = softplus(rho) * eps              [U, E]
    out[b,u] = sum_e gw[b,e] * ( (x @ mu[:,:,e])[b,u] + s[b]*se[u,e] + bias[u,e] )
    with s[b] = sum_d x[b,d].

Sharding: data-parallel over batch. Each of the 8 cores processes 512 rows
of x and produces its 512-row slice of the output; the host concatenates.
No collectives needed.

Structure (v2):
  - Expert weights mu are scaled by 512, cast to fp8e4 (TRN E4M3, max 240)
    on the host, and kept RESIDENT in SBUF (8 MB) — loaded once outside the
    iteration loop, so steady-state HBM traffic is just x in / y out.
  - Mean-path matmuls run in fp8 DoubleRow perf mode. Within each output
    tile the experts are processed in two half-groups of 4 with the k-pair
    loop outermost, so each 256-row stationary tile is amortized over 4
    matmuls instead of being reloaded per matmul.
  - The noise+bias terms are accumulated by the PE into a per-tile seed
    PSUM (two tiny matmuls against (gw*s)^T and gw^T), so the final
    combine is just the 1+8 DVE drain chain; no dense noise matrices.
  - Gating stays in fp32r on an exact fp32 copy of x.
  - Per hardware-loop trip, two iterations are emitted as
    head(A) head(B) mean(A) mean(B) with double-buffered tiles, so B's
    gating runs on DVE/ACT/PE while A's mean matmuls stream, and the
    next iteration's x DMA overlaps the previous mean phase.

Measured end-to-end relative error vs a float64 reference: ~8.4e-3
(fp8 quantization of x and mu; tolerance is 2e-2).
"""

import numpy as np
import ml_dtypes

import concourse.bass as bass
from concourse import bacc
import concourse.mybir as mybir
import concourse.tile as tile
from concourse.bass_utils import run_bass_kernel_spmd
from concourse.masks import make_identity

N_CORES = 8
B, D, U, E = 4096, 1024, 1024, 8
P = 128                 # partitions
BS = B // N_CORES       # 512 batch rows per core
KT = D // P             # 8 contraction subtiles
KP = KT // 2            # 4 DoubleRow contraction pairs
BT = BS // P            # 4 batch tiles per core
NT = 512                # matmul moving free dim (one PSUM bank of fp32)
UT = U // NT            # 2 output column tiles
MU_SCALE = 512.0        # fp8 range scaling for mu (|mu*512| < 40 << 240)

F32 = mybir.dt.float32
F32R = mybir.dt.float32r
F8 = mybir.dt.float8e4
BF16 = mybir.dt.bfloat16
F8NP = ml_dtypes.float8_e4m3
AF = mybir.ActivationFunctionType
ALU = mybir.AluOpType
DR = mybir.MatmulPerfMode.DoubleRow

_CACHE: dict = {}


def _consts(nc, tc, cp, t):
    """Load loop-invariant tensors into SBUF and precompute se = softplus(rho)*eps."""
    C = {}
    C["mu8"] = cp.tile([P, E, KT, U], F8, name="mu8")
    nc.sync.dma_start(
        out=C["mu8"], in_=t["muR8"].rearrange("e (kt p) u -> p e kt u", p=P)
    )
    C["gk"] = cp.tile([P, KT, 10], F32R, name="gk_sb")
    nc.sync.dma_start(out=C["gk"], in_=t["gk"].rearrange("(kt p) e -> p kt e", p=P))
    C["gb"] = cp.tile([1, 10], F32R, name="gb_sb")
    nc.sync.dma_start(out=C["gb"], in_=t["gb"])
    rho = cp.tile([E, U], F32)
    nc.sync.dma_start(out=rho, in_=t["rhoT"])
    eps = cp.tile([E, U], F32)
    nc.sync.dma_start(out=eps, in_=t["epsT"])
    C["ones1"] = cp.tile([1, P], F32R, name="ones1")
    nc.sync.dma_start(out=C["ones1"], in_=t["onesd"])
    C["ident"] = cp.tile([P, P], F32, name="ident")
    make_identity(nc, C["ident"])

    # seb16: rows 0:8 = softplus(rho)*eps (pairs with (gw*s)^T), rows 8:16 =
    # expert bias (pairs with gw^T) — one 16-deep seed matmul. Compute ops
    # stay on partitions 0:8 (aligned); only DMA touches rows 8:16.
    # softplus as ln(1 + exp(rho)); rho ~ -2.6 so no overflow.
    C["seb16"] = cp.tile([16, U], F32, name="seb16")
    nc.sync.dma_start(out=C["seb16"][8:16, :], in_=t["biasT"])
    nc.scalar.activation(out=C["seb16"][0:8, :], in_=rho, func=AF.Exp)
    nc.scalar.activation(
        out=C["seb16"][0:8, :], in_=C["seb16"][0:8, :], func=AF.Ln, bias=1.0
    )
    nc.vector.tensor_mul(C["seb16"][0:8, :], C["seb16"][0:8, :], eps)
    return C


def _head(nc, tc, C, ip, pp, t):
    """Load x, compute gating. Returns the per-iteration tiles _mean() needs."""
    it = {}
    xt = ip.tile([P, KT, BS], F32R, tag="xt", bufs=2)
    nc.sync.dma_start(out=xt, in_=t["xT"].rearrange("(kt p) b -> p kt b", p=P))
    x8 = ip.tile([P, KT, BS], F8, tag="x8", bufs=2)
    nc.sync.dma_start(out=x8, in_=t["x8T"].rearrange("(kt p) b -> p kt b", p=P))
    it["x8"] = x8

    # gw16 cols 0:8 = gates * rowsum(x); cols 8:16 = renormalized top-2 gates
    gw16 = ip.tile([P, BT, 16], F32, tag="gw16", bufs=2)
    gws = ip.tile([P, BT, E], F32, tag="gws", bufs=2)    # gw / MU_SCALE
    gwT16 = ip.tile([16, BS], F32, tag="gwT16", bufs=2)  # transposed gates
    it["gws"] = gws
    it["gwT16"] = gwT16

    for bt in range(BT):
        pg = pp.tile([P, 10], F32, tag="gat", bufs=2)
        for kt in range(KT):
            nc.tensor.matmul(
                pg,
                lhsT=xt[:, kt, bt * P:(bt + 1) * P],
                rhs=C["gk"][:, kt, :],
                start=(kt == 0),
                stop=False,
            )
        # add gating bias (and 0 for the row-sum column): ones^T x gb_row
        nc.tensor.matmul(pg, lhsT=C["ones1"], rhs=C["gb"], start=False, stop=True)

        logit = pg[:, 0:8]
        m1 = ip.tile([P, 1], F32, tag="m1", bufs=2)
        nc.vector.tensor_reduce(out=m1, in_=logit, axis=mybir.AxisListType.X, op=ALU.max)
        mask = ip.tile([P, 8], F32, tag="mask", bufs=2)
        nc.vector.tensor_scalar(out=mask, in0=logit, scalar1=m1, scalar2=None, op0=ALU.is_equal)
        l2 = ip.tile([P, 8], F32, tag="l2", bufs=2)
        nc.vector.scalar_tensor_tensor(
            out=l2, in0=mask, scalar=-1e30, in1=logit, op0=ALU.mult, op1=ALU.add
        )
        m2 = ip.tile([P, 1], F32, tag="m2", bufs=2)
        nc.vector.tensor_reduce(out=m2, in_=l2, axis=mybir.AxisListType.X, op=ALU.max)
        nc.vector.tensor_scalar(out=mask, in0=logit, scalar1=m2, scalar2=None, op0=ALU.is_ge)

        el = ip.tile([P, 8], F32, tag="el", bufs=2)
        nc.scalar.activation(out=el, in_=logit, func=AF.Exp)
        gm = ip.tile([P, 8], F32, tag="gm", bufs=2)
        den = ip.tile([P, 1], F32, tag="den", bufs=2)
        nc.vector.scalar_tensor_tensor(
            out=gm, in0=el, scalar=1.0, in1=mask, op0=ALU.mult, op1=ALU.mult, accum_out=den
        )
        inv = ip.tile([P, 1], F32, tag="inv", bufs=2)
        nc.vector.reciprocal(inv, den)
        nc.vector.tensor_scalar_mul(gw16[:, bt, 8:16], gm, inv)
        nc.vector.tensor_scalar(
            out=gws[:, bt, :], in0=gw16[:, bt, 8:16], scalar1=1.0 / MU_SCALE,
            scalar2=None, op0=ALU.mult,
        )
        s = ip.tile([P, 1], F32, tag="s", bufs=2)
        nc.scalar.copy(s, pg[:, 8:9])
        nc.vector.tensor_scalar_mul(gw16[:, bt, 0:8], gw16[:, bt, 8:16], s)

        # transpose [gw*s | gw] to (e, b) for the seed matmuls
        pt = pp.tile([16, P], F32, tag="gat", bufs=2)
        nc.tensor.transpose(pt, gw16[:, bt, :], C["ident"])
        nc.scalar.copy(gwT16[:, bt * P:(bt + 1) * P], pt)
    return it


def _mean(nc, tc, C, ip, pp, t, it):
    """fp8 DoubleRow expert matmuls + PSUM-seeded noise/bias + drain chain."""
    x8, gws, gwT16 = it["x8"], it["gws"], it["gwT16"]
    for bt in range(BT):
        bn = slice(bt * P, (bt + 1) * P)
        uns = [slice(ut * NT, (ut + 1) * NT) for ut in range(UT)]
        # seed = sum_e gw*bias + sum_e (gw*s)*se, one 16-deep matmul per ut
        yas = []
        for ut in range(UT):
            sps = pp.tile([P, NT], F32, tag="seed", bufs=2)
            nc.tensor.matmul(
                sps, lhsT=gwT16[:, bn], rhs=C["seb16"][:, uns[ut]],
                start=True, stop=True,
            )
            ya = ip.tile([P, NT], F32, tag="ya", bufs=4)
            nc.vector.tensor_copy(out=ya, in_=sps)
            yas.append(ya)

        outs = [None, None]
        for e in range(E):
            pss = [
                pp.tile([P, NT], F32, tag="ps", bufs=4, name=f"ps_{bt}_{e}_{ut}")
                for ut in range(UT)
            ]
            # ut innermost: each 256-row stationary tile serves both column
            # tiles before the next LoadStationary
            for kp in range(KP):
                for ut in range(UT):
                    nc.tensor.matmul(
                        pss[ut],
                        lhsT=x8[:, 2 * kp:2 * kp + 2, bn],
                        rhs=C["mu8"][:, e, 2 * kp:2 * kp + 2, uns[ut]],
                        start=(kp == 0),
                        stop=(kp == KP - 1),
                        perf_mode=DR,
                    )
            gwe = gws[:, bt, e:e + 1]
            for ut in range(UT):
                out = yas[ut]
                if e == E - 1:
                    out = ip.tile([P, NT], F32, tag="yo", bufs=3)
                    outs[ut] = out
                nc.vector.scalar_tensor_tensor(
                    out=out, in0=pss[ut], scalar=gwe, in1=yas[ut],
                    op0=ALU.mult, op1=ALU.add,
                )
        for ut in range(UT):
            nc.sync.dma_start(out=t["y"][bn, uns[ut]], in_=outs[ut])


def _body_pair(nc, tc, C, ip, pp, t):
    itA = _head(nc, tc, C, ip, pp, t)
    itB = _head(nc, tc, C, ip, pp, t)
    _mean(nc, tc, C, ip, pp, t, itA)
    _mean(nc, tc, C, ip, pp, t, itB)


def build(reps=1):
    key = ("nc", reps)
    if key in _CACHE:
        return _CACHE[key]
    nc = bacc.Bacc("TRN2", target_bir_lowering=False)
    t = {
        "xT": nc.dram_tensor("xT", [D, BS], F32R, kind="ExternalInput").ap(),
        "x8T": nc.dram_tensor("x8T", [D, BS], F8, kind="ExternalInput").ap(),
        "muR8": nc.dram_tensor("muR8", [E, D, U], F8, kind="ExternalInput").ap(),
        "gk": nc.dram_tensor("gk", [D, 10], F32R, kind="ExternalInput").ap(),
        "gb": nc.dram_tensor("gb", [1, 10], F32R, kind="ExternalInput").ap(),
        "rhoT": nc.dram_tensor("rhoT", [E, U], F32, kind="ExternalInput").ap(),
        "biasT": nc.dram_tensor("biasT", [E, U], F32, kind="ExternalInput").ap(),
        "epsT": nc.dram_tensor("epsT", [E, U], F32, kind="ExternalInput").ap(),
        "onesd": nc.dram_tensor("onesd", [1, P], F32R, kind="ExternalInput").ap(),
        "y": nc.dram_tensor("y", [BS, U], F32, kind="ExternalOutput").ap(),
    }
    with tile.TileContext(nc) as tc:
        with tc.tile_pool(name="const", bufs=1) as cp:
            C = _consts(nc, tc, cp, t)
            with (
                tc.tile_pool(name="iter", bufs=1) as ip,
                tc.tile_pool(name="psum", bufs=1, space="PSUM") as pp,
            ):
                if reps == 1:
                    it = _head(nc, tc, C, ip, pp, t)
                    _mean(nc, tc, C, ip, pp, t, it)
                else:
                    assert reps % 2 == 0, "loop body is emitted twice per trip"
                    with tc.For_i(0, reps // 2, 1):
                        _body_pair(nc, tc, C, ip, pp, t)
    nc.compile()
    _CACHE[key] = nc
    return nc


def prep_inputs(x, expert_mu, expert_rho, expert_bias, gating_kernel, gating_bias, eps):
    """Host-side sharding / layout prep (transpose, dtype cast, replication)."""
    x = np.ascontiguousarray(np.asarray(x, dtype=np.float32))
    mu = np.asarray(expert_mu, dtype=np.float32)        # [D, U, E]
    # e-major fp8 weights, scaled into the TRN E4M3 normal range
    muR8 = np.ascontiguousarray(
        np.clip(np.transpose(mu, (2, 0, 1)) * MU_SCALE, -240.0, 240.0)
    ).astype(F8NP)
    gk = np.concatenate(
        [np.asarray(gating_kernel, dtype=np.float32), np.ones((D, 1), np.float32),
         np.zeros((D, 1), np.float32)], axis=1
    )  # [D, 10]: col 8 computes the row-sums s; col 9 pads to even width (fp32r ISA)
    gb = np.concatenate(
        [np.asarray(gating_bias, dtype=np.float32), np.zeros((2,), np.float32)]
    ).reshape(1, 10)
    rhoT = np.ascontiguousarray(np.asarray(expert_rho, dtype=np.float32).T)  # [E, U]
    epsT = np.ascontiguousarray(np.asarray(eps, dtype=np.float32).T)         # [E, U]
    biasT = np.ascontiguousarray(np.asarray(expert_bias, dtype=np.float32).T)
    shared = {"muR8": muR8, "gk": gk, "gb": gb, "rhoT": rhoT, "epsT": epsT,
              "biasT": biasT, "onesd": np.ones((1, P), np.float32)}
    in_maps = []
    for cid in range(N_CORES):
        xs = np.ascontiguousarray(x[cid * BS:(cid + 1) * BS].T)  # [D, BS]
        in_maps.append({"xT": xs, "x8T": xs.astype(F8NP), **shared})
    return in_maps


def kernel(x, expert_mu, expert_rho, expert_bias, gating_kernel, gating_bias, eps, k):
    assert int(k) == 2, f"kernel is specialized for top-2 gating, got k={k}"
    nc = build()
    in_maps = prep_inputs(
        x, expert_mu, expert_rho, expert_bias, gating_kernel, gating_bias, eps
    )
    res = run_bass_kernel_spmd(nc, in_maps, list(range(N_CORES)))
    return np.concatenate([res.results[c]["y"] for c in range(N_CORES)], axis=0)


# revision 16
# speedup vs baseline: 1.1996x; 1.1996x over previous
"""Bayesian dense MoE (top-2 of 8 experts) on 8 Trainium2 NeuronCores.

Math (per reference):
    logits = x @ gk + gb                      [B, E]
    gw     = renorm-top2(softmax(logits))     [B, E]   (softmax denom cancels)
    se     = softplus(rho) * eps              [U, E]
    out[b,u] = sum_e gw[b,e] * ( (x @ mu[:,:,e])[b,u] + s[b]*se[u,e] + bias[u,e] )
    with s[b] = sum_d x[b,d].

Sharding: data-parallel over batch. Each of the 8 cores processes 512 rows
of x and produces its 512-row slice of the output; the host concatenates.
No collectives needed.

Structure (v2):
  - Expert weights mu are scaled by 512, cast to fp8e4 (TRN E4M3, max 240)
    on the host, and kept RESIDENT in SBUF (8 MB) — loaded once outside the
    iteration loop, so steady-state HBM traffic is just x in / y out.
  - Mean-path matmuls run in fp8 DoubleRow perf mode. Within each output
    tile the experts are processed in two half-groups of 4 with the k-pair
    loop outermost, so each 256-row stationary tile is amortized over 4
    matmuls instead of being reloaded per matmul.
  - The noise+bias terms are accumulated by the PE into a per-tile seed
    PSUM (two tiny matmuls against (gw*s)^T and gw^T), so the final
    combine is just the 1+8 DVE drain chain; no dense noise matrices.
  - Gating stays in fp32r on an exact fp32 copy of x.
  - Per hardware-loop trip, two iterations are emitted as
    head(A) head(B) mean(A) mean(B) with double-buffered tiles, so B's
    gating runs on DVE/ACT/PE while A's mean matmuls stream, and the
    next iteration's x DMA overlaps the previous mean phase.

Measured end-to-end relative error vs a float64 reference: ~8.4e-3
(fp8 quantization of x and mu; tolerance is 2e-2).
"""

import numpy as np
import ml_dtypes

import concourse.bass as bass
from concourse import bacc
import concourse.mybir as mybir
import concourse.tile as tile
from concourse.bass_utils import run_bass_kernel_spmd
from concourse.masks import make_identity

N_CORES = 8
B, D, U, E = 4096, 1024, 1024, 8
P = 128                 # partitions
BS = B // N_CORES       # 512 batch rows per core
KT = D // P             # 8 contraction subtiles
KP = KT // 2            # 4 DoubleRow contraction pairs
BT = BS // P            # 4 batch tiles per core
NT = 512                # matmul moving free dim (one PSUM bank of fp32)
UT = U // NT            # 2 output column tiles
MU_SCALE = 512.0        # fp8 range scaling for mu (|mu*512| < 40 << 240)

F32 = mybir.dt.float32
F32R = mybir.dt.float32r
F8 = mybir.dt.float8e4
BF16 = mybir.dt.bfloat16
F8NP = ml_dtypes.float8_e4m3
AF = mybir.ActivationFunctionType
ALU = mybir.AluOpType
DR = mybir.MatmulPerfMode.DoubleRow

_CACHE: dict = {}


def _consts(nc, tc, cp, t):
    """Load loop-invariant tensors into SBUF and precompute se = softplus(rho)*eps."""
    C = {}
    C["mu8"] = cp.tile([P, E, KT, U], F8, name="mu8")
    nc.sync.dma_start(
        out=C["mu8"], in_=t["muR8"].rearrange("e (kt p) u -> p e kt u", p=P)
    )
    C["gk"] = cp.tile([P, KT, 10], F32R, name="gk_sb")
    nc.sync.dma_start(out=C["gk"], in_=t["gk"].rearrange("(kt p) e -> p kt e", p=P))
    C["gb"] = cp.tile([1, 10], F32R, name="gb_sb")
    nc.sync.dma_start(out=C["gb"], in_=t["gb"])
    rho = cp.tile([E, U], F32)
    nc.sync.dma_start(out=rho, in_=t["rhoT"])
    eps = cp.tile([E, U], F32)
    nc.sync.dma_start(out=eps, in_=t["epsT"])
    C["ones1"] = cp.tile([1, P], F32R, name="ones1")
    nc.sync.dma_start(out=C["ones1"], in_=t["onesd"])
    C["ident"] = cp.tile([P, P], F32, name="ident")
    make_identity(nc, C["ident"])

    # seb16: rows 0:8 = softplus(rho)*eps (pairs with (gw*s)^T), rows 8:16 =
    # expert bias (pairs with gw^T) — one 16-deep seed matmul. Compute ops
    # stay on partitions 0:8 (aligned); only DMA touches rows 8:16.
    # softplus as ln(1 + exp(rho)); rho ~ -2.6 so no overflow.
    C["seb16"] = cp.tile([16, U], F32, name="seb16")
    nc.sync.dma_start(out=C["seb16"][8:16, :], in_=t["biasT"])
    nc.scalar.activation(out=C["seb16"][0:8, :], in_=rho, func=AF.Exp)
    nc.scalar.activation(
        out=C["seb16"][0:8, :], in_=C["seb16"][0:8, :], func=AF.Ln, bias=1.0
    )
    nc.vector.tensor_mul(C["seb16"][0:8, :], C["seb16"][0:8, :], eps)
    return C


def _head(nc, tc, C, ip, pp, t, eng=None):
    """Load x, compute gating. Returns the per-iteration tiles _mean() needs."""
    eng = eng or nc.sync
    it = {}
    xt = ip.tile([P, KT, BS], F32R, tag="xt", bufs=2)
    eng.dma_start(out=xt, in_=t["xT"].rearrange("(kt p) b -> p kt b", p=P))
    x8 = ip.tile([P, KT, BS], F8, tag="x8", bufs=2)
    eng.dma_start(out=x8, in_=t["x8T"].rearrange("(kt p) b -> p kt b", p=P))
    it["x8"] = x8

    # gw16 cols 0:8 = gates * rowsum(x); cols 8:16 = renormalized top-2 gates
    gw16 = ip.tile([P, BT, 16], F32, tag="gw16", bufs=2)
    gws = ip.tile([P, BT, E], F32, tag="gws", bufs=2)    # gw / MU_SCALE
    gwT16 = ip.tile([16, BS], F32, tag="gwT16", bufs=2)  # transposed gates
    it["gws"] = gws
    it["gwT16"] = gwT16

    for bt in range(BT):
        pg = pp.tile([P, 10], F32, tag="gat", bufs=2)
        for kt in range(KT):
            nc.tensor.matmul(
                pg,
                lhsT=xt[:, kt, bt * P:(bt + 1) * P],
                rhs=C["gk"][:, kt, :],
                start=(kt == 0),
                stop=False,
            )
        # add gating bias (and 0 for the row-sum column): ones^T x gb_row
        nc.tensor.matmul(pg, lhsT=C["ones1"], rhs=C["gb"], start=False, stop=True)

        logit = pg[:, 0:8]
        m1 = ip.tile([P, 1], F32, tag="m1", bufs=2)
        nc.vector.tensor_reduce(out=m1, in_=logit, axis=mybir.AxisListType.X, op=ALU.max)
        mask = ip.tile([P, 8], F32, tag="mask", bufs=2)
        nc.vector.tensor_scalar(out=mask, in0=logit, scalar1=m1, scalar2=None, op0=ALU.is_equal)
        l2 = ip.tile([P, 8], F32, tag="l2", bufs=2)
        nc.vector.scalar_tensor_tensor(
            out=l2, in0=mask, scalar=-1e30, in1=logit, op0=ALU.mult, op1=ALU.add
        )
        m2 = ip.tile([P, 1], F32, tag="m2", bufs=2)
        nc.vector.tensor_reduce(out=m2, in_=l2, axis=mybir.AxisListType.X, op=ALU.max)
        nc.vector.tensor_scalar(out=mask, in0=logit, scalar1=m2, scalar2=None, op0=ALU.is_ge)

        el = ip.tile([P, 8], F32, tag="el", bufs=2)
        nc.scalar.activation(out=el, in_=logit, func=AF.Exp)
        gm = ip.tile([P, 8], F32, tag="gm", bufs=2)
        den = ip.tile([P, 1], F32, tag="den", bufs=2)
        nc.vector.scalar_tensor_tensor(
            out=gm, in0=el, scalar=1.0, in1=mask, op0=ALU.mult, op1=ALU.mult, accum_out=den
        )
        inv = ip.tile([P, 1], F32, tag="inv", bufs=2)
        nc.vector.reciprocal(inv, den)
        nc.vector.tensor_scalar_mul(gw16[:, bt, 8:16], gm, inv)
        nc.vector.tensor_scalar(
            out=gws[:, bt, :], in0=gw16[:, bt, 8:16], scalar1=1.0 / MU_SCALE,
            scalar2=None, op0=ALU.mult,
        )
        s = ip.tile([P, 1], F32, tag="s", bufs=2)
        nc.scalar.copy(s, pg[:, 8:9])
        nc.vector.tensor_scalar_mul(gw16[:, bt, 0:8], gw16[:, bt, 8:16], s)

        # transpose [gw*s | gw] to (e, b) for the seed matmuls
        pt = pp.tile([16, P], F32, tag="gat", bufs=2)
        nc.tensor.transpose(pt, gw16[:, bt, :], C["ident"])
        nc.scalar.copy(gwT16[:, bt * P:(bt + 1) * P], pt)
    return it


def _mean(nc, tc, C, ip, pp, t, it, eng=None):
    """fp8 DoubleRow expert matmuls + PSUM-seeded noise/bias + drain chain."""
    eng = eng or nc.sync
    x8, gws, gwT16 = it["x8"], it["gws"], it["gwT16"]
    for bt in range(BT):
        bn = slice(bt * P, (bt + 1) * P)
        uns = [slice(ut * NT, (ut + 1) * NT) for ut in range(UT)]
        # seed = sum_e gw*bias + sum_e (gw*s)*se, one 16-deep matmul per ut
        yas = []
        for ut in range(UT):
            sps = pp.tile([P, NT], F32, tag="seed", bufs=2)
            nc.tensor.matmul(
                sps, lhsT=gwT16[:, bn], rhs=C["seb16"][:, uns[ut]],
                start=True, stop=True,
            )
            ya = ip.tile([P, NT], F32, tag="ya", bufs=4)
            nc.vector.tensor_copy(out=ya, in_=sps)
            yas.append(ya)

        outs = [None, None]
        for e in range(E):
            pss = [
                pp.tile([P, NT], F32, tag="ps", bufs=4, name=f"ps_{bt}_{e}_{ut}")
                for ut in range(UT)
            ]
            # ut innermost: each 256-row stationary tile serves both column
            # tiles before the next LoadStationary
            for kp in range(KP):
                for ut in range(UT):
                    nc.tensor.matmul(
                        pss[ut],
                        lhsT=x8[:, 2 * kp:2 * kp + 2, bn],
                        rhs=C["mu8"][:, e, 2 * kp:2 * kp + 2, uns[ut]],
                        start=(kp == 0),
                        stop=(kp == KP - 1),
                        perf_mode=DR,
                    )
            gwe = gws[:, bt, e:e + 1]
            for ut in range(UT):
                out = yas[ut]
                if e == E - 1:
                    out = ip.tile([P, NT], F32, tag="yo", bufs=3)
                    outs[ut] = out
                nc.vector.scalar_tensor_tensor(
                    out=out, in0=pss[ut], scalar=gwe, in1=yas[ut],
                    op0=ALU.mult, op1=ALU.add,
                )
        for ut in range(UT):
            eng.dma_start(out=t["y"][bn, uns[ut]], in_=outs[ut])


def _body_quad(nc, tc, C, ip, pp, t):
    # 4-deep pipeline per trip: heads C/D are emitted between the mean
    # phases so their DMA + gating hide under earlier means; x loads and y
    # stores alternate between the SP and ACT HWDGE rings so concurrent
    # slots' transfers run in parallel.
    itA = _head(nc, tc, C, ip, pp, t, nc.sync)
    itB = _head(nc, tc, C, ip, pp, t, nc.scalar)
    _mean(nc, tc, C, ip, pp, t, itA, nc.scalar)
    itC = _head(nc, tc, C, ip, pp, t, nc.sync)
    _mean(nc, tc, C, ip, pp, t, itB, nc.sync)
    itD = _head(nc, tc, C, ip, pp, t, nc.scalar)
    _mean(nc, tc, C, ip, pp, t, itC, nc.scalar)
    _mean(nc, tc, C, ip, pp, t, itD, nc.sync)


def build(reps=1):
    key = ("nc", reps)
    if key in _CACHE:
        return _CACHE[key]
    nc = bacc.Bacc("TRN2", target_bir_lowering=False)
    t = {
        "xT": nc.dram_tensor("xT", [D, BS], F32R, kind="ExternalInput").ap(),
        "x8T": nc.dram_tensor("x8T", [D, BS], F8, kind="ExternalInput").ap(),
        "muR8": nc.dram_tensor("muR8", [E, D, U], F8, kind="ExternalInput").ap(),
        "gk": nc.dram_tensor("gk", [D, 10], F32R, kind="ExternalInput").ap(),
        "gb": nc.dram_tensor("gb", [1, 10], F32R, kind="ExternalInput").ap(),
        "rhoT": nc.dram_tensor("rhoT", [E, U], F32, kind="ExternalInput").ap(),
        "biasT": nc.dram_tensor("biasT", [E, U], F32, kind="ExternalInput").ap(),
        "epsT": nc.dram_tensor("epsT", [E, U], F32, kind="ExternalInput").ap(),
        "onesd": nc.dram_tensor("onesd", [1, P], F32R, kind="ExternalInput").ap(),
        "y": nc.dram_tensor("y", [BS, U], F32, kind="ExternalOutput").ap(),
    }
    with tile.TileContext(nc) as tc:
        with tc.tile_pool(name="const", bufs=1) as cp:
            C = _consts(nc, tc, cp, t)
            with (
                tc.tile_pool(name="iter", bufs=1) as ip,
                tc.tile_pool(name="psum", bufs=1, space="PSUM") as pp,
            ):
                if reps == 1:
                    it = _head(nc, tc, C, ip, pp, t)
                    _mean(nc, tc, C, ip, pp, t, it)
                else:
                    assert reps % 4 == 0, "loop body is emitted 4x per trip"
                    with tc.For_i(0, reps // 4, 1):
                        _body_quad(nc, tc, C, ip, pp, t)
    nc.compile()
    _CACHE[key] = nc
    return nc


def prep_inputs(x, expert_mu, expert_rho, expert_bias, gating_kernel, gating_bias, eps):
    """Host-side sharding / layout prep (transpose, dtype cast, replication)."""
    x = np.ascontiguousarray(np.asarray(x, dtype=np.float32))
    mu = np.asarray(expert_mu, dtype=np.float32)        # [D, U, E]
    # e-major fp8 weights, scaled into the TRN E4M3 normal range
    muR8 = np.ascontiguousarray(
        np.clip(np.transpose(mu, (2, 0, 1)) * MU_SCALE, -240.0, 240.0)
    ).astype(F8NP)
    gk = np.concatenate(
        [np.asarray(gating_kernel, dtype=np.float32), np.ones((D, 1), np.float32),
         np.zeros((D, 1), np.float32)], axis=1
    )  # [D, 10]: col 8 computes the row-sums s; col 9 pads to even width (fp32r ISA)
    gb = np.concatenate(
        [np.asarray(gating_bias, dtype=np.float32), np.zeros((2,), np.float32)]
    ).reshape(1, 10)
    rhoT = np.ascontiguousarray(np.asarray(expert_rho, dtype=np.float32).T)  # [E, U]
    epsT = np.ascontiguousarray(np.asarray(eps, dtype=np.float32).T)         # [E, U]
    biasT = np.ascontiguousarray(np.asarray(expert_bias, dtype=np.float32).T)
    shared = {"muR8": muR8, "gk": gk, "gb": gb, "rhoT": rhoT, "epsT": epsT,
              "biasT": biasT, "onesd": np.ones((1, P), np.float32)}
    in_maps = []
    for cid in range(N_CORES):
        xs = np.ascontiguousarray(x[cid * BS:(cid + 1) * BS].T)  # [D, BS]
        in_maps.append({"xT": xs, "x8T": xs.astype(F8NP), **shared})
    return in_maps


def kernel(x, expert_mu, expert_rho, expert_bias, gating_kernel, gating_bias, eps, k):
    assert int(k) == 2, f"kernel is specialized for top-2 gating, got k={k}"
    nc = build()
    in_maps = prep_inputs(
        x, expert_mu, expert_rho, expert_bias, gating_kernel, gating_bias, eps
    )
    res = run_bass_kernel_spmd(nc, in_maps, list(range(N_CORES)))
    return np.concatenate([res.results[c]["y"] for c in range(N_CORES)], axis=0)
